# revision 18
# baseline (speedup 1.0000x reference)
"""Stereo cost-volume generator (nn_CostGenerator) for 8 Trainium2 cores.

Iter-1 control build: baseline structure (3 out-DMA splits per channel on
the Sync ring, GpSimd wedge memsets, inputs on the Scalar ring) with all
on-device traffic in bfloat16.
"""

import numpy as np

B, C, H, W, D = 2, 32, 128, 256, 48
NCORES = 8
CH = (B * C) // NCORES  # channels per core
PW = W + D - 1  # padded right row: 47 zeros + 256 values
NG = D // 8  # 6 groups of 8 e-rows
WK = [216 + 8 * k for k in range(NG)]  # group widths
GOFF = [0]
for k in range(NG):
    GOFF.append(GOFF[-1] + 8 * WK[k])
PACK = GOFF[-1]  # 11328 elems per partition
# out-DMA split points: groups [0-1], [2-3], [4-5]
OUT_SPLITS = [(GOFF[0], GOFF[2]), (GOFF[2], GOFF[4]), (GOFF[4], GOFF[6])]


def _cap(ap, base_off, part_pitch, dims):
    """Custom AP on ap's tensor at ap.offset+base_off; partition dim [pitch, H],
    free dims = list of (stride, size)."""
    import bass_rust

    return bass_rust.AP(
        tensor=ap.tensor,
        offset=ap.offset + base_off,
        ap=bass_rust.VecI64Pair([[part_pitch, H]] + [list(d) for d in dims]),
    )


def _build_nc():
    import concourse.bacc as bacc
    import concourse.mybir as mybir
    from concourse.tile import TileContext

    bf16 = mybir.dt.bfloat16
    nc = bacc.Bacc()
    inp = nc.declare_dram_parameter("inp", [2, CH, H, W], bf16, isOutput=False)
    out = nc.declare_dram_parameter("out", [CH, H, PACK], bf16, isOutput=True)

    with TileContext(nc) as tc:
        with tc.tile_pool(name="io", bufs=1) as pool:
            lt = pool.tile([H, CH * W], bf16, tag="lt", name="lt")
            rp = pool.tile([H, CH * PW], bf16, tag="rp", name="rp")
            obufs = [
                pool.tile([H, PACK], bf16, tag=f"ob{i}", name=f"ob{i}")
                for i in range(3)
            ]

            # zero the 47-col pad strips of all right channels (one 2D memset)
            nc.vector.memset(_cap(rp, 0, CH * PW, [(PW, CH), (1, D - 1)]), 0.0)

            # channel-0 inputs first so compute can start early; all input
            # loads go on the Sync HWDGE ring; the output streams via the
            # Scalar ring (whose queue-master engine runs at full rate).
            nc.sync.dma_start(out=lt[:, :W], in_=inp[0][0])
            nc.sync.dma_start(
                out=_cap(rp, D - 1, CH * PW, [(1, W)]), in_=inp[1][0]
            )
            # remaining channels
            nc.sync.dma_start(
                out=_cap(lt, W, CH * W, [(W, CH - 1), (1, W)]),
                in_=inp[0][1:].transpose([1, 0, 2]),
            )
            nc.sync.dma_start(
                out=_cap(rp, PW + D - 1, CH * PW, [(PW, CH - 1), (1, W)]),
                in_=inp[1][1:].transpose([1, 0, 2]),
            )

            for j in range(CH):
                ob = obufs[j % 3]
                for k in range(NG):
                    wk, w0 = WK[k], 40 - 8 * k
                    # ob[h, G_k + i*wk + t] = left[h, w0+t] - rpad[h, 40+i+t]
                    nc.vector.tensor_sub(
                        out=_cap(ob, GOFF[k], PACK, [(wk, 8), (1, wk)]),
                        in0=_cap(lt, j * W + w0, CH * W, [(0, 8), (1, wk)]),
                        in1=_cap(rp, j * PW + 40, CH * PW, [(1, 8), (1, wk)]),
                    )
                # re-zero garbage cells: group k, row i<7, cols [0, 7-i)
                for k in range(NG):
                    for i in range(7):
                        o = GOFF[k] + i * WK[k]
                        nc.gpsimd.memset(ob[:, o : o + 7 - i], 0.0)
                for a, b in OUT_SPLITS:
                    nc.scalar.dma_start(out=out[j][:, a:b], in_=ob[:, a:b])
    nc.finalize()
    return nc


def _shard_inputs(left_feature, right_feature):
    import ml_dtypes

    bf16 = ml_dtypes.bfloat16
    lf = np.asarray(left_feature, dtype=np.float32).astype(bf16).reshape(B * C, H, W)
    rf = np.asarray(right_feature, dtype=np.float32).astype(bf16).reshape(B * C, H, W)
    in_maps = []
    for i in range(NCORES):
        sl = slice(i * CH, (i + 1) * CH)
        in_maps.append({"inp": np.ascontiguousarray(np.stack([lf[sl], rf[sl]]))})
    return in_maps


def _unpack_core(arr):
    # arr: [CH, H, PACK] packed bf16 -> [CH, D, H, W] dense f32 (d-order)
    cost = np.zeros((arr.shape[0], D, H, W), np.float32)
    for k in range(NG):
        wk, w0 = WK[k], 40 - 8 * k
        blk = arr[:, :, GOFF[k] : GOFF[k + 1]].reshape(arr.shape[0], H, 8, wk)
        for i in range(8):
            d = D - 1 - (8 * k + i)
            cost[:, d, :, w0:] = blk[:, :, i, :]
    return cost


def _gather(results):
    parts = [_unpack_core(np.asarray(r["out"])) for r in results]
    cost = np.concatenate(parts, axis=0).reshape(B, C, D, H, W)
    return np.ascontiguousarray(cost)


def kernel(left_feature, right_feature, max_disp_at_scale):
    assert int(max_disp_at_scale) == D, max_disp_at_scale
    from concourse.bass_utils import run_bass_kernel_spmd

    nc = _build_nc()
    in_maps = _shard_inputs(left_feature, right_feature)
    res = run_bass_kernel_spmd(nc, in_maps, core_ids=list(range(NCORES)))
    return _gather(res.results)


# revision 19
# speedup vs baseline: 1.1347x; 1.1347x over previous
"""Stereo cost-volume generator (nn_CostGenerator) for 8 Trainium2 cores.

bf16 build, outputs streamed via the GpSimd software DGE (qPoolDynamic):
the HWDGE queue-master engine (E79) loses ~20% throughput to descriptor
bookkeeping on whichever HW queue carries the heavy stream, pacing the
whole kernel; SWDGE generates descriptors on the Q7 cores instead.
"""

import numpy as np

B, C, H, W, D = 2, 32, 128, 256, 48
NCORES = 8
CH = (B * C) // NCORES  # channels per core
PW = W + D - 1  # padded right row: 47 zeros + 256 values
NG = D // 8  # 6 groups of 8 e-rows
WK = [216 + 8 * k for k in range(NG)]  # group widths
GOFF = [0]
for k in range(NG):
    GOFF.append(GOFF[-1] + 8 * WK[k])
PACK = GOFF[-1]  # 11328 elems per partition
# out-DMA split points: groups [0-1], [2-3], [4-5]
OUT_SPLITS = [(GOFF[0], GOFF[2]), (GOFF[2], GOFF[4]), (GOFF[4], GOFF[6])]


def _cap(ap, base_off, part_pitch, dims):
    """Custom AP on ap's tensor at ap.offset+base_off; partition dim [pitch, H],
    free dims = list of (stride, size)."""
    import bass_rust

    return bass_rust.AP(
        tensor=ap.tensor,
        offset=ap.offset + base_off,
        ap=bass_rust.VecI64Pair([[part_pitch, H]] + [list(d) for d in dims]),
    )


def _build_nc():
    import concourse.bacc as bacc
    import concourse.mybir as mybir
    from concourse.tile import TileContext

    bf16 = mybir.dt.bfloat16
    nc = bacc.Bacc()
    inp = nc.declare_dram_parameter("inp", [2, CH, H, W], bf16, isOutput=False)
    out = nc.declare_dram_parameter("out", [CH, H, PACK], bf16, isOutput=True)

    with TileContext(nc) as tc:
        with tc.tile_pool(name="io", bufs=1) as pool:
            lt = pool.tile([H, CH * W], bf16, tag="lt", name="lt")
            rp = pool.tile([H, CH * PW], bf16, tag="rp", name="rp")
            obufs = [
                pool.tile([H, PACK], bf16, tag=f"ob{i}", name=f"ob{i}")
                for i in range(3)
            ]

            # zero the 47-col pad strips of all right channels (one 2D memset)
            nc.vector.memset(_cap(rp, 0, CH * PW, [(PW, CH), (1, D - 1)]), 0.0)

            # channel-0 inputs first so compute can start early; all input
            # loads go on the Sync HWDGE ring; the output streams via the
            # Scalar ring (whose queue-master engine runs at full rate).
            nc.sync.dma_start(out=lt[:, :W], in_=inp[0][0])
            nc.sync.dma_start(
                out=_cap(rp, D - 1, CH * PW, [(1, W)]), in_=inp[1][0]
            )
            # remaining channels
            nc.sync.dma_start(
                out=_cap(lt, W, CH * W, [(W, CH - 1), (1, W)]),
                in_=inp[0][1:].transpose([1, 0, 2]),
            )
            nc.sync.dma_start(
                out=_cap(rp, PW + D - 1, CH * PW, [(PW, CH - 1), (1, W)]),
                in_=inp[1][1:].transpose([1, 0, 2]),
            )

            for j in range(CH):
                ob = obufs[j % 3]
                for k in range(NG):
                    wk, w0 = WK[k], 40 - 8 * k
                    # ob[h, G_k + i*wk + t] = left[h, w0+t] - rpad[h, 40+i+t]
                    nc.vector.tensor_sub(
                        out=_cap(ob, GOFF[k], PACK, [(wk, 8), (1, wk)]),
                        in0=_cap(lt, j * W + w0, CH * W, [(0, 8), (1, wk)]),
                        in1=_cap(rp, j * PW + 40, CH * PW, [(1, 8), (1, wk)]),
                    )
                # garbage cells (group k, row i<7, cols [0,7-i)) are not
                # zeroed on device; the host unpack skips them.
                for a, b in OUT_SPLITS:
                    nc.gpsimd.dma_start(out=out[j][:, a:b], in_=ob[:, a:b])
    nc.finalize()
    return nc


def _shard_inputs(left_feature, right_feature):
    import ml_dtypes

    bf16 = ml_dtypes.bfloat16
    lf = np.asarray(left_feature, dtype=np.float32).astype(bf16).reshape(B * C, H, W)
    rf = np.asarray(right_feature, dtype=np.float32).astype(bf16).reshape(B * C, H, W)
    in_maps = []
    for i in range(NCORES):
        sl = slice(i * CH, (i + 1) * CH)
        in_maps.append({"inp": np.ascontiguousarray(np.stack([lf[sl], rf[sl]]))})
    return in_maps


def _unpack_core(arr):
    # arr: [CH, H, PACK] packed bf16 -> [CH, D, H, W] dense f32 (d-order).
    # Row i of group k holds disparity d = 47-(8k+i); its first 7-i cells are
    # garbage (Hankel window overlapping the zero pad) and the valid region
    # of disparity d starts at w = d, so copy cols [7-i:] only.
    cost = np.zeros((arr.shape[0], D, H, W), np.float32)
    for k in range(NG):
        wk, w0 = WK[k], 40 - 8 * k
        blk = arr[:, :, GOFF[k] : GOFF[k + 1]].reshape(arr.shape[0], H, 8, wk)
        for i in range(8):
            d = D - 1 - (8 * k + i)
            s = max(0, 7 - i)
            cost[:, d, :, w0 + s :] = blk[:, :, i, s:]
    return cost


def _gather(results):
    parts = [_unpack_core(np.asarray(r["out"])) for r in results]
    cost = np.concatenate(parts, axis=0).reshape(B, C, D, H, W)
    return np.ascontiguousarray(cost)


def kernel(left_feature, right_feature, max_disp_at_scale):
    assert int(max_disp_at_scale) == D, max_disp_at_scale
    from concourse.bass_utils import run_bass_kernel_spmd

    nc = _build_nc()
    in_maps = _shard_inputs(left_feature, right_feature)
    res = run_bass_kernel_spmd(nc, in_maps, core_ids=list(range(NCORES)))
    return _gather(res.results)


# revision 22
# speedup vs baseline: 1.1473x; 1.0111x over previous
"""Stereo cost-volume generator (nn_CostGenerator) for 8 Trainium2 cores.

bf16 build, outputs streamed via the GpSimd software DGE (qPoolDynamic):
the HWDGE queue-master engine (E79) loses ~20% throughput to descriptor
bookkeeping on whichever HW queue carries the heavy stream, pacing the
whole kernel; SWDGE generates descriptors on the Q7 cores instead.
"""

import numpy as np

B, C, H, W, D = 2, 32, 128, 256, 48
NCORES = 8
CH = (B * C) // NCORES  # channels per core
PW = W + D - 1  # padded right row: 47 zeros + 256 values
NG = D // 8  # 6 groups of 8 e-rows
WK = [216 + 8 * k for k in range(NG)]  # group widths
GOFF = [0]
for k in range(NG):
    GOFF.append(GOFF[-1] + 8 * WK[k])
PACK = GOFF[-1]  # 11328 elems per partition
# out-DMA split points: channel 0 streams per-group so the queue starts
# draining as early as possible; later channels per group-pair.
SPLITS_CH0 = [(GOFF[k], GOFF[k + 1]) for k in range(NG)]
OUT_SPLITS = [(GOFF[0], GOFF[2]), (GOFF[2], GOFF[4]), (GOFF[4], GOFF[6])]


def _cap(ap, base_off, part_pitch, dims):
    """Custom AP on ap's tensor at ap.offset+base_off; partition dim [pitch, H],
    free dims = list of (stride, size)."""
    import bass_rust

    return bass_rust.AP(
        tensor=ap.tensor,
        offset=ap.offset + base_off,
        ap=bass_rust.VecI64Pair([[part_pitch, H]] + [list(d) for d in dims]),
    )


def _build_nc():
    import concourse.bacc as bacc
    import concourse.mybir as mybir
    from concourse.tile import TileContext

    bf16 = mybir.dt.bfloat16
    nc = bacc.Bacc()
    inp = nc.declare_dram_parameter("inp", [2, CH, H, W], bf16, isOutput=False)
    out = nc.declare_dram_parameter("out", [CH, H, PACK], bf16, isOutput=True)

    with TileContext(nc) as tc:
        with tc.tile_pool(name="io", bufs=1) as pool:
            lt = pool.tile([H, CH * W], bf16, tag="lt", name="lt")
            rp = pool.tile([H, CH * PW], bf16, tag="rp", name="rp")
            obufs = [
                pool.tile([H, PACK], bf16, tag=f"ob{i}", name=f"ob{i}")
                for i in range(3)
            ]

            # zero the 47-col pad strips of all right channels (one 2D memset)
            nc.vector.memset(_cap(rp, 0, CH * PW, [(PW, CH), (1, D - 1)]), 0.0)

            # channel-0 inputs first so compute can start early; all input
            # loads go on the Sync HWDGE ring, away from the SWDGE output
            # stream.
            nc.sync.dma_start(out=lt[:, :W], in_=inp[0][0])
            nc.sync.dma_start(
                out=_cap(rp, D - 1, CH * PW, [(1, W)]), in_=inp[1][0]
            )
            # remaining channels
            nc.sync.dma_start(
                out=_cap(lt, W, CH * W, [(W, CH - 1), (1, W)]),
                in_=inp[0][1:].transpose([1, 0, 2]),
            )
            nc.sync.dma_start(
                out=_cap(rp, PW + D - 1, CH * PW, [(PW, CH - 1), (1, W)]),
                in_=inp[1][1:].transpose([1, 0, 2]),
            )

            for j in range(CH):
                ob = obufs[j % 3]
                for k in range(NG):
                    wk, w0 = WK[k], 40 - 8 * k
                    # ob[h, G_k + i*wk + t] = left[h, w0+t] - rpad[h, 40+i+t]
                    nc.vector.tensor_sub(
                        out=_cap(ob, GOFF[k], PACK, [(wk, 8), (1, wk)]),
                        in0=_cap(lt, j * W + w0, CH * W, [(0, 8), (1, wk)]),
                        in1=_cap(rp, j * PW + 40, CH * PW, [(1, 8), (1, wk)]),
                    )
                # garbage cells (group k, row i<7, cols [0,7-i)) are not
                # zeroed on device; the host unpack skips them.
                for a, b in SPLITS_CH0 if j == 0 else OUT_SPLITS:
                    nc.gpsimd.dma_start(out=out[j][:, a:b], in_=ob[:, a:b])
    nc.finalize()
    return nc


def _shard_inputs(left_feature, right_feature):
    import ml_dtypes

    bf16 = ml_dtypes.bfloat16
    lf = np.asarray(left_feature, dtype=np.float32).astype(bf16).reshape(B * C, H, W)
    rf = np.asarray(right_feature, dtype=np.float32).astype(bf16).reshape(B * C, H, W)
    in_maps = []
    for i in range(NCORES):
        sl = slice(i * CH, (i + 1) * CH)
        in_maps.append({"inp": np.ascontiguousarray(np.stack([lf[sl], rf[sl]]))})
    return in_maps


def _unpack_core(arr):
    # arr: [CH, H, PACK] packed bf16 -> [CH, D, H, W] dense f32 (d-order).
    # Row i of group k holds disparity d = 47-(8k+i); its first 7-i cells are
    # garbage (Hankel window overlapping the zero pad) and the valid region
    # of disparity d starts at w = d, so copy cols [7-i:] only.
    cost = np.zeros((arr.shape[0], D, H, W), np.float32)
    for k in range(NG):
        wk, w0 = WK[k], 40 - 8 * k
        blk = arr[:, :, GOFF[k] : GOFF[k + 1]].reshape(arr.shape[0], H, 8, wk)
        for i in range(8):
            d = D - 1 - (8 * k + i)
            s = max(0, 7 - i)
            cost[:, d, :, w0 + s :] = blk[:, :, i, s:]
    return cost


def _gather(results):
    parts = [_unpack_core(np.asarray(r["out"])) for r in results]
    cost = np.concatenate(parts, axis=0).reshape(B, C, D, H, W)
    return np.ascontiguousarray(cost)


def kernel(left_feature, right_feature, max_disp_at_scale):
    assert int(max_disp_at_scale) == D, max_disp_at_scale
    from concourse.bass_utils import run_bass_kernel_spmd

    nc = _build_nc()
    in_maps = _shard_inputs(left_feature, right_feature)
    res = run_bass_kernel_spmd(nc, in_maps, core_ids=list(range(NCORES)))
    return _gather(res.results)
